# revision 1
# baseline (speedup 1.0000x reference)
"""Trainium2 Bass kernel: GCN message passing (nn_DDI_gcn), 8 NeuronCores SPMD.

Math:
  agg[r] = sum_{e: row_idx[e]==r} vals[e] * mEmbed[col_idx[e] % 50000]
  out[i] = 2*(inter*relu(agg[i]) + (1-inter)*relu(agg[i+50000])),  i < 50000

Strategy (destination sharding, no cross-core reduction):
  * Core k owns output rows [6272k, 6272(k+1)); host buckets every edge by
    (core, 128-row dest tile, table half, plane) and pads each bucket run to
    a 128-edge chunk boundary. Edges are sorted by gather address within each
    run (HBM locality) and pad slots carry index -1.
  * Device, per dest tile ("group"): four+ dma_gathers (flat chunk list
    split evenly over 4 SWDGE queues for parallel descriptor processing)
    fetch the edges' embedding rows (fp16, 256 B/row) into SBUF. For each
    128-edge chunk build the selection matrix S[e, r] = val[e]*(d[e]==r) with
    ONE dual-op tensor_scalar (is_equal -> mult) from a constant iota tile;
    TensorE accumulates S^T @ G into a per-plane PSUM tile (the segment sum);
    epilogue applies a*relu(psumA) + b*relu(psumB) and streams the 128x128
    f32 tile out.
  * All index math is host-side numpy; the device never touches raw indices
    except as dma_gather int16 offsets.
"""

import numpy as np

import concourse.bass as bass
import concourse.bacc as bacc
import concourse.tile as tile
import concourse.mybir as mybir
from concourse.bass_utils import run_bass_kernel_spmd

MED = 50000
NCORES = 8
TILES = 49               # dest tiles per plane per core
RPC = TILES * 128        # 6272 dest rows per core (per plane)
HALF = 25000             # equal table halves: balanced SWDGE queue loads
                         # (any split <= 32768 keeps gather idxs in int16)
P = 128
F = 128                  # feature dim

_NC_CACHE = {}


def build_nc(C0, C1, tiles=TILES, gbufs=5, repeat=1):
    """C0/C1: chunks per half0/half1 run. Group chunk layout: [A0|B0|A1|B1].
    repeat>1 re-runs the whole body (timing: marginal = pure HW time)."""
    CG = 2 * C0 + 2 * C1
    N0 = 2 * C0 * 128    # idxs in table half 0
    N1 = 2 * C1 * 128
    IC = (N0 + N1) // 16
    dt16 = mybir.dt.float16
    f32 = mybir.dt.float32

    nc = bacc.Bacc(None, target_bir_lowering=False, num_swdge_queues=4)
    table = nc.dram_tensor("table", [MED, F], dt16, kind="ExternalInput")
    idx_d = nc.dram_tensor("idx", [tiles, P, IC], mybir.dt.int16, kind="ExternalInput")
    dval_d = nc.dram_tensor("dval", [tiles, P, 2 * CG], f32, kind="ExternalInput")
    ab_d = nc.dram_tensor("ab", [P, 2], f32, kind="ExternalInput")
    iota_d = nc.dram_tensor("iota", [P, P], dt16, kind="ExternalInput")
    out_d = nc.dram_tensor("out", [tiles, P, F], f32, kind="ExternalOutput")

    planes = [0] * C0 + [1] * C0 + [0] * C1 + [1] * C1
    firstA, lastA = 0, 2 * C0 + C1 - 1
    firstB, lastB = C0, CG - 1

    # One gather call per run (A0,B0,A1,B1), one SWDGE queue each. The call
    # pattern must be 4-periodic so the round-robin DMASW sem lanes (8) stay
    # consistently locked to their queues.
    # (chunk_lo, chunk_hi, queue)
    calls = [
        (0, C0, 0),
        (C0, 2 * C0, 1),
        (2 * C0, 2 * C0 + C1, 2),
        (2 * C0 + C1, CG, 3),
    ]

    def idx_cols(c):  # idx column of chunk c's first index
        return 8 * c if c <= 2 * C0 else N0 // 16 + 8 * (c - 2 * C0)

    with tile.TileContext(nc) as tc:
        with (
            tc.tile_pool(name="const", bufs=1) as constp,
            tc.tile_pool(name="gbuf", bufs=gbufs) as gbufp,
            tc.tile_pool(name="meta", bufs=4) as metap,
            tc.tile_pool(name="sp", bufs=16) as sp,
            tc.tile_pool(name="ep", bufs=6) as ep,
            tc.tile_pool(name="psum", bufs=4, space=bass.MemorySpace.PSUM) as psp,
        ):
            iota_t = constp.tile([P, P], dt16, tag="iota")
            nc.sync.dma_start(iota_t[:], iota_d[:])
            ab_t = constp.tile([P, 2], f32, tag="ab")
            nc.sync.dma_start(ab_t[:], ab_d[:])

            for gi, g in enumerate(
                [g_ for _ in range(repeat) for g_ in range(tiles)]
            ):
                idx_t = metap.tile([P, IC], mybir.dt.int16, tag="idx")
                nc.sync.dma_start(idx_t[:], idx_d[g])
                dv_t = metap.tile([P, 2 * CG], f32, tag="dval")
                nc.sync.dma_start(dv_t[:], dval_d[g])

                g_t = gbufp.tile([P, CG, F], dt16, tag="g")
                for (clo, chi, q) in calls:
                    n = (chi - clo) * 128
                    tlo, thi = (0, HALF) if chi <= 2 * C0 else (HALF, MED)
                    nc.gpsimd.dma_gather(
                        g_t[:, clo:chi, :], table[tlo:thi, :],
                        idx_t[:, idx_cols(clo) : idx_cols(chi)], n, n, F,
                        single_packet=False, queue_num=q,
                    )

                psA = psp.tile([P, F], f32, tag="psA")
                psB = psp.tile([P, F], f32, tag="psB")
                for c in range(CG):
                    s_t = sp.tile([P, P], dt16, tag="s")
                    nc.vector.tensor_scalar(
                        s_t[:], iota_t[:],
                        dv_t[:, c : c + 1], dv_t[:, CG + c : CG + c + 1],
                        mybir.AluOpType.is_equal, mybir.AluOpType.mult,
                    )
                    if planes[c] == 0:
                        nc.tensor.matmul(psA[:], s_t[:], g_t[:, c, :],
                                         start=(c == firstA), stop=(c == lastA))
                    else:
                        nc.tensor.matmul(psB[:], s_t[:], g_t[:, c, :],
                                         start=(c == firstB), stop=(c == lastB))

                t0 = ep.tile([P, F], f32, tag="t0")
                nc.vector.tensor_scalar(t0[:], psA[:], 0.0, ab_t[:, 0:1],
                                        mybir.AluOpType.max, mybir.AluOpType.mult)
                t1 = ep.tile([P, F], f32, tag="t1")
                nc.vector.tensor_scalar(t1[:], psB[:], 0.0, ab_t[:, 1:2],
                                        mybir.AluOpType.max, mybir.AluOpType.mult)
                o_t = ep.tile([P, F], f32, tag="o")
                nc.vector.tensor_tensor(o_t[:], t0[:], t1[:], mybir.AluOpType.add)
                nc.sync.dma_start(out_d[g], o_t[:])

    nc.compile()
    return nc


def preprocess(vals, mEmbed, inter, row_idx, col_idx, tiles=TILES):
    E = row_idx.shape[0]
    col = col_idx.astype(np.int64) % MED
    rowl = row_idx.astype(np.int64)
    plane = rowl // MED
    prow = rowl % MED
    core = np.minimum(prow // RPC, NCORES - 1)
    lt = (prow - core * RPC) >> 7
    d = (prow & 127).astype(np.float32)
    half = (col >= HALF).astype(np.int64)
    lidx = (col - half * HALF).astype(np.int16)

    run = half * 2 + plane                      # A0,B0,A1,B1 order
    key = (core * tiles + lt) * 4 + run
    order = np.lexsort((lidx, key))             # addr-sorted within run
    ksort = key[order]
    nk = NCORES * tiles * 4
    cnt = np.bincount(ksort, minlength=nk)
    starts = np.concatenate([[0], np.cumsum(cnt)[:-1]])
    rank = np.arange(E, dtype=np.int64) - starts[ksort]

    cnt4 = cnt.reshape(-1, 4)
    C0 = max(1, int(np.ceil(cnt4[:, 0:2].max() / 128)))
    C1 = max(1, int(np.ceil(cnt4[:, 2:4].max() / 128)))
    CG = 2 * C0 + 2 * C1
    N0 = 2 * C0 * 128
    N1 = 2 * C1 * 128
    run_off = np.array([0, C0 * 128, 2 * C0 * 128, (2 * C0 + C1) * 128])
    SLOTS_G = CG * 128
    gidx = ksort // 4
    slot = gidx * SLOTS_G + run_off[ksort % 4] + rank
    TOT = NCORES * tiles * SLOTS_G

    IDX = np.zeros(TOT, np.int16)
    VAL = np.zeros(TOT, np.float32)
    DD = np.zeros(TOT, np.float32)
    IDX[slot] = lidx[order]
    VAL[slot] = np.asarray(vals, np.float32)[order]
    DD[slot] = d[order]

    IDX4 = IDX.reshape(NCORES, tiles, CG, 128)
    i0 = (IDX4[:, :, : 2 * C0, :].reshape(NCORES, tiles, N0 // 16, 16)
          .transpose(0, 1, 3, 2))
    i1 = (IDX4[:, :, 2 * C0 :, :].reshape(NCORES, tiles, N1 // 16, 16)
          .transpose(0, 1, 3, 2))
    idx16 = np.concatenate([i0, i1], axis=3)           # [NC, tiles, 16, IC]
    idx128 = np.ascontiguousarray(np.tile(idx16, (1, 1, 8, 1)))

    D4 = DD.reshape(NCORES, tiles, CG, 128).transpose(0, 1, 3, 2)
    V4 = VAL.reshape(NCORES, tiles, CG, 128).transpose(0, 1, 3, 2)
    dval = np.ascontiguousarray(np.concatenate([D4, V4], axis=3), dtype=np.float32)

    table16 = np.asarray(mEmbed, np.float32).astype(np.float16)
    iota = np.ascontiguousarray(
        np.broadcast_to(np.arange(128, dtype=np.float16), (128, 128)))
    a = 2.0 * np.float32(np.asarray(inter).reshape(-1)[0])
    b = np.float32(2.0) - a
    ab = np.ascontiguousarray(
        np.stack([np.full(128, a, np.float32), np.full(128, b, np.float32)], axis=1))
    return C0, C1, table16, iota, ab, idx128, dval


def _in_maps(pre):
    C0, C1, table16, iota, ab, idx128, dval = pre
    return [
        {"table": table16, "iota": iota, "ab": ab,
         "idx": idx128[k], "dval": dval[k]}
        for k in range(NCORES)
    ]


def _run(vals, mEmbed, inter, row_idx, col_idx, trace=False):
    pre = preprocess(vals, mEmbed, inter, row_idx, col_idx)
    C0, C1 = pre[0], pre[1]
    key = (C0, C1, 1)
    if key not in _NC_CACHE:
        _NC_CACHE[key] = build_nc(C0, C1)
    nc = _NC_CACHE[key]
    res = run_bass_kernel_spmd(nc, _in_maps(pre), core_ids=list(range(NCORES)),
                               trace=trace)
    full = np.concatenate(
        [res.results[k]["out"].reshape(RPC, F) for k in range(NCORES)], axis=0)
    return np.ascontiguousarray(full[:MED]), res


def kernel(vals, mEmbed, inter, row_idx, col_idx):
    out, _ = _run(vals, mEmbed, inter, row_idx, col_idx, trace=False)
    return out


def _make_sharded(nc, donate=False):
    """Replicate bass2jax.run_bass_via_pjrt's executable construction so we
    can reuse it for repeated timed executions."""
    import jax
    from jax.sharding import Mesh, PartitionSpec
    from jax.experimental.shard_map import shard_map
    from concourse import bass2jax as b2j

    b2j.install_neuronx_cc_hook()
    partition_name = nc.partition_id_tensor.name if nc.partition_id_tensor else None
    in_names, out_names, out_avals, zero_outs = [], [], [], []
    for alloc in nc.m.functions[0].allocations:
        if not isinstance(alloc, mybir.MemoryLocationSet):
            continue
        name = alloc.memorylocations[0].name
        if alloc.kind == "ExternalInput":
            if name != partition_name:
                in_names.append(name)
        elif alloc.kind == "ExternalOutput":
            out_names.append(name)
            shape = tuple(alloc.tensor_shape)
            dtype = mybir.dt.np(alloc.dtype)
            out_avals.append(jax.core.ShapedArray(shape, dtype))
            zero_outs.append(np.zeros(shape, dtype))
    n_params = len(in_names)
    in_names = in_names + out_names
    if partition_name is not None:
        in_names = in_names + [partition_name]

    def _body(*args):
        operands = list(args)
        if partition_name is not None:
            operands.append(b2j.partition_id_tensor())
        outs = b2j._bass_exec_p.bind(
            *operands,
            out_avals=tuple(out_avals),
            in_names=tuple(in_names),
            out_names=tuple(out_names),
            lowering_input_output_aliases=(),
            sim_require_finite=True,
            sim_require_nnan=True,
            nc=nc,
        )
        return tuple(outs)

    devices = jax.devices()[:NCORES]
    mesh = Mesh(np.asarray(devices), ("core",))
    in_specs = (PartitionSpec("core"),) * (n_params + len(out_names))
    out_specs = (PartitionSpec("core"),) * len(out_names)
    kw = dict(donate_argnums=tuple(range(n_params, n_params + len(out_names)))) if donate else {}

    sharded = jax.jit(
        shard_map(_body, mesh=mesh, in_specs=in_specs,
                  out_specs=out_specs, check_rep=False),
        keep_unused=True, **kw)
    return sharded, mesh, in_names[:n_params], out_names, zero_outs


def timed_run(vals, mEmbed, inter, row_idx, col_idx, k=9, samples=12,
              build_kwargs=None):
    """Time on device: build the same program with the body repeated 1x and
    kx INSIDE the NEFF; marginal = (median T(k) - median T(1)) / (k-1) =
    pure HW time (per-call dispatch overhead and tunnel latency cancel)."""
    import time
    import jax
    from jax.sharding import NamedSharding, PartitionSpec

    pre = preprocess(vals, mEmbed, inter, row_idx, col_idx)
    C0, C1 = pre[0], pre[1]
    bk = dict(build_kwargs or {})
    per_core = _in_maps(pre)

    def make_runner(repeat):
        ck = (C0, C1, repeat, tuple(sorted(bk.items())))
        if ck not in _NC_CACHE:
            _NC_CACHE[ck] = build_nc(C0, C1, repeat=repeat, **bk)
        nc = _NC_CACHE[ck]
        sharded, mesh, in_names, out_names, zero_outs = _make_sharded(nc)
        sh = NamedSharding(mesh, PartitionSpec("core"))
        concat_in = [
            jax.device_put(
                np.concatenate([np.asarray(per_core[c][n]) for c in range(NCORES)],
                               axis=0), sh)
            for n in in_names
        ]
        concat_zero = [
            jax.device_put(np.zeros((NCORES * z.shape[0], *z.shape[1:]), z.dtype), sh)
            for z in zero_outs
        ]
        def run():
            out = sharded(*concat_in, *concat_zero)
            jax.block_until_ready(out)
        run()   # warm-up / compile
        return run

    r1 = make_runner(1)
    rk = make_runner(k)

    # Model switches cost ~10 ms through the tunnel, so time each NEFF in
    # bursts of consecutive calls (drop the first two after each switch) and
    # alternate bursts to cancel slow drift in the per-call overhead.
    def burst(run, n=6, discard=2):
        ts = []
        for _ in range(n):
            t0 = time.perf_counter(); run(); ts.append(time.perf_counter() - t0)
        return ts[discard:]

    t1s, tks = [], []
    for _ in range(max(6, samples // 2)):
        t1s += burst(r1)
        tks += burst(rk)
    t1 = float(np.median(t1s))
    tk = float(np.median(tks))
    marginal_ns = (tk - t1) / (k - 1) * 1e9
    return int(marginal_ns), int(t1 * 1e9), int(tk * 1e9)



# revision 35
# speedup vs baseline: 1.2595x; 1.2595x over previous
"""Trainium2 Bass kernel: GCN message passing (nn_DDI_gcn), 8 NeuronCores SPMD.

Math:
  agg[r] = sum_{e: row_idx[e]==r} vals[e] * mEmbed[col_idx[e] % 50000]
  out[i] = 2*(inter*relu(agg[i]) + (1-inter)*relu(agg[i+50000])),  i < 50000

Strategy (destination sharding, no cross-core reduction):
  * Core k owns output rows [6272k, 6272(k+1)); host buckets every edge by
    (core, 128-row dest tile, table half, plane) and pads each bucket run to
    a 128-edge chunk boundary. Edges are sorted by gather address within each
    run (HBM locality) and pad slots carry index -1.
  * Device, per dest tile ("group"): four+ dma_gathers (flat chunk list
    split evenly over 4 SWDGE queues for parallel descriptor processing)
    fetch the edges' embedding rows (fp16, 256 B/row) into SBUF. For each
    128-edge chunk build the selection matrix S[e, r] = val[e]*(d[e]==r) with
    ONE dual-op tensor_scalar (is_equal -> mult) from a constant iota tile;
    TensorE accumulates S^T @ G into a per-plane PSUM tile (the segment sum);
    epilogue applies a*relu(psumA) + b*relu(psumB) and streams the 128x128
    f32 tile out.
  * All index math is host-side numpy; the device never touches raw indices
    except as dma_gather int16 offsets.
"""

import numpy as np

import concourse.bass as bass
import concourse.bacc as bacc
import concourse.tile as tile
import concourse.mybir as mybir
from concourse.bass_utils import run_bass_kernel_spmd

AF = mybir.ActivationFunctionType

MED = 50000
NCORES = 8
TILES = 49               # dest tiles per plane per core
RPC = TILES * 128        # 6272 dest rows per core (per plane)
HALF = 25000             # equal table halves: balanced SWDGE queue loads
                         # (any split <= 32768 keeps gather idxs in int16)
P = 128
F = 128                  # feature dim

_NC_CACHE = {}


def build_nc(C0, C1, tiles=TILES, gbufs=7, repeat=1,
             do_gather=True, do_compute=True, probe=None, scratch=16384,
             cstride=1, actmod=0, sp1=False, dvespam=0, merge_calls=False):
    """C0/C1: chunks per half0/half1 run. Group chunk layout: [A0|B0|A1|B1].
    repeat>1 re-runs the whole body (timing: marginal = pure HW time).
    do_gather/do_compute: microbenchmark switches (timing only).
    probe: None | 'seqdma' (replace gather w/ sequential DMA of same volume)
         | 'e2x' (gather 512B descs, same desc count, 2x bytes)
         | 'q1'  (all gathers on one SWDGE queue).
    cstride: compute only every cstride-th chunk (timing probe)."""
    CG = 2 * C0 + 2 * C1
    N0 = 2 * C0 * 128    # idxs in table half 0
    N1 = 2 * C1 * 128
    IC = (N0 + N1) // 16
    dt16 = mybir.dt.float16
    f32 = mybir.dt.float32

    nc = bacc.Bacc(None, target_bir_lowering=False, num_swdge_queues=4,
                   dynamic_dma_scratch_size=scratch)
    table = nc.dram_tensor("table", [MED, F], dt16, kind="ExternalInput")
    # dval layout: [d | v | -v] (3*CG columns); -v feeds the Act-engine
    # S-build (scale=-v, bias=+v).
    idx32_d = (nc.dram_tensor("idx32", [tiles, P, CG], mybir.dt.int32,
                              kind="ExternalInput")
               if probe in ("ind", "hybrid") else None)
    t2x = (nc.dram_tensor("t2x", [HALF, 2 * F], dt16, kind="ExternalInput")
           if probe == "e2x" else None)
    tbig = (nc.dram_tensor("tbig", [P, CG, F], dt16, kind="ExternalInput")
            if probe == "seqdma" else None)
    idx_d = nc.dram_tensor("idx", [tiles, P, IC], mybir.dt.int16, kind="ExternalInput")
    dval_d = nc.dram_tensor("dval", [tiles, P, 3 * CG], f32, kind="ExternalInput")
    ab_d = nc.dram_tensor("ab", [P, 2], f32, kind="ExternalInput")
    iota_d = nc.dram_tensor("iota", [P, P], dt16, kind="ExternalInput")
    out_d = nc.dram_tensor("out", [tiles, P, F], f32, kind="ExternalOutput")

    planes = [0] * C0 + [1] * C0 + [0] * C1 + [1] * C1
    firstA, lastA = 0, 2 * C0 + C1 - 1
    firstB, lastB = C0, CG - 1

    # One gather call per run (A0,B0,A1,B1), one SWDGE queue each. The call
    # pattern must be 4-periodic so the round-robin DMASW sem lanes (8) stay
    # consistently locked to their queues.
    # (chunk_lo, chunk_hi, queue)
    calls = [
        (0, C0, 0),
        (C0, 2 * C0, 1),
        (2 * C0, 2 * C0 + C1, 2),
        (2 * C0 + C1, CG, 3),
    ]

    def idx_cols(c):  # idx column of chunk c's first index
        return 8 * c if c <= 2 * C0 else N0 // 16 + 8 * (c - 2 * C0)

    with tile.TileContext(nc) as tc:
        with (
            tc.tile_pool(name="const", bufs=1) as constp,
            tc.tile_pool(name="gbuf", bufs=gbufs) as gbufp,
            tc.tile_pool(name="meta", bufs=4) as metap,
            tc.tile_pool(name="sp", bufs=16) as sp,
            tc.tile_pool(name="ep", bufs=6) as ep,
            tc.tile_pool(name="psum", bufs=4, space=bass.MemorySpace.PSUM) as psp,
        ):
            iota_t = constp.tile([P, P], dt16, tag="iota")
            nc.sync.dma_start(iota_t[:], iota_d[:])
            ab_t = constp.tile([P, 2], f32, tag="ab")
            nc.sync.dma_start(ab_t[:], ab_d[:])

            for gi, g in enumerate(
                [g_ for _ in range(repeat) for g_ in range(tiles)]
            ):
                idx_t = metap.tile([P, IC], mybir.dt.int16, tag="idx")
                nc.sync.dma_start(idx_t[:], idx_d[g])
                dv_t = metap.tile([P, 3 * CG], f32, tag="dval")
                nc.sync.dma_start(dv_t[:], dval_d[g])

                gw = 2 * F if probe == "e2x" else F
                g_t = gbufp.tile([P, CG, gw], dt16, tag="g")
                if probe in ("ind", "hybrid"):
                    i32_t = metap.tile([P, CG], mybir.dt.int32, tag="i32")
                    nc.sync.dma_start(i32_t[:], idx32_d[g])
                if probe == "seqdma":
                    nc.sync.dma_start(g_t[:], tbig[:])
                elif probe == "ind":
                    nc.gpsimd.indirect_dma_start(
                        g_t[:, :, :], None, table[:, :],
                        bass.IndirectOffsetOnAxis(ap=i32_t[:, :], axis=0),
                    )
                elif probe == "hybrid":
                    # runs A0,B0 via SWDGE queues 0-1; A1,B1 via indirect
                    for (clo, chi, q) in calls[:2]:
                        n = (chi - clo) * 128
                        nc.gpsimd.dma_gather(
                            g_t[:, clo:chi, :], table[0:HALF, :],
                            idx_t[:, idx_cols(clo) : idx_cols(chi)], n, n, F,
                            single_packet=sp1, queue_num=q,
                        )
                    nc.gpsimd.indirect_dma_start(
                        g_t[:, 2 * C0 :, :], None, table[:, :],
                        bass.IndirectOffsetOnAxis(ap=i32_t[:, 2 * C0 :], axis=0),
                    )
                elif do_gather and merge_calls:
                    # one call per table half; queues alternate by group
                    for j, (clo, chi) in enumerate([(0, 2 * C0), (2 * C0, CG)]):
                        n = (chi - clo) * 128
                        tlo, thi = (0, HALF) if j == 0 else (HALF, MED)
                        nc.gpsimd.dma_gather(
                            g_t[:, clo:chi, :], table[tlo:thi, :],
                            idx_t[:, idx_cols(clo) : idx_cols(chi)], n, n, F,
                            single_packet=False,
                            queue_num=2 * (g % 2) + j,
                        )
                elif do_gather:
                    for (clo, chi, q) in calls:
                        n = (chi - clo) * 128
                        if probe == "e2x":
                            nc.gpsimd.dma_gather(
                                g_t[:, clo:chi, :], t2x[:, :],
                                idx_t[:, idx_cols(clo) : idx_cols(chi)], n, n,
                                2 * F, single_packet=False, queue_num=q,
                            )
                            continue
                        tlo, thi = (0, HALF) if chi <= 2 * C0 else (HALF, MED)
                        nc.gpsimd.dma_gather(
                            g_t[:, clo:chi, :], table[tlo:thi, :],
                            idx_t[:, idx_cols(clo) : idx_cols(chi)], n, n, F,
                            single_packet=sp1,
                            queue_num=0 if probe == "q1" else q,
                        )
                for _ in range(dvespam):
                    sp_t = sp.tile([P, P], dt16, tag="spam")
                    nc.vector.tensor_scalar(
                        sp_t[:], iota_t[:], 1.0, 2.0,
                        mybir.AluOpType.mult, mybir.AluOpType.add)
                if not do_compute:
                    continue

                psA = psp.tile([P, F], f32, tag="psA")
                psB = psp.tile([P, F], f32, tag="psB")
                kept = [c for c in range(CG) if c % cstride == 0]
                kA = [c for c in kept if planes[c] == 0]
                kB = [c for c in kept if planes[c] == 1]
                for ci, c in enumerate(kept):
                    s_t = sp.tile([P, P], dt16, tag="s")
                    if actmod and ci % actmod == actmod - 1:
                        # Act-engine S-build: |j-d| then relu(v - v*|j-d|)
                        a1 = sp.tile([P, P], dt16, tag="a1")
                        nc.scalar.activation(
                            a1[:], iota_t[:], AF.Abs,
                            bias=dv_t[:, c : c + 1], scale=-1.0)
                        nc.scalar.activation(
                            s_t[:], a1[:], AF.Relu,
                            bias=dv_t[:, CG + c : CG + c + 1],
                            scale=dv_t[:, 2 * CG + c : 2 * CG + c + 1])
                    else:
                        nc.vector.tensor_scalar(
                            s_t[:], iota_t[:],
                            dv_t[:, c : c + 1], dv_t[:, CG + c : CG + c + 1],
                            mybir.AluOpType.is_equal, mybir.AluOpType.mult,
                        )
                    if planes[c] == 0:
                        nc.tensor.matmul(psA[:], s_t[:], g_t[:, c, :],
                                         start=(c == kA[0]), stop=(c == kA[-1]))
                    else:
                        nc.tensor.matmul(psB[:], s_t[:], g_t[:, c, :],
                                         start=(c == kB[0]), stop=(c == kB[-1]))

                t0 = ep.tile([P, F], f32, tag="t0")
                nc.vector.tensor_scalar(t0[:], psA[:], 0.0, ab_t[:, 0:1],
                                        mybir.AluOpType.max, mybir.AluOpType.mult)
                t1 = ep.tile([P, F], f32, tag="t1")
                nc.vector.tensor_scalar(t1[:], psB[:], 0.0, ab_t[:, 1:2],
                                        mybir.AluOpType.max, mybir.AluOpType.mult)
                o_t = ep.tile([P, F], f32, tag="o")
                nc.vector.tensor_tensor(o_t[:], t0[:], t1[:], mybir.AluOpType.add)
                nc.sync.dma_start(out_d[g], o_t[:])

    nc.compile()
    return nc


def preprocess(vals, mEmbed, inter, row_idx, col_idx, tiles=TILES):
    E = row_idx.shape[0]
    col = col_idx.astype(np.int64) % MED
    rowl = row_idx.astype(np.int64)
    plane = rowl // MED
    prow = rowl % MED
    core = np.minimum(prow // RPC, NCORES - 1)
    lt = (prow - core * RPC) >> 7
    d = (prow & 127).astype(np.float32)
    half = (col >= HALF).astype(np.int64)
    lidx = (col - half * HALF).astype(np.int16)

    run = half * 2 + plane                      # A0,B0,A1,B1 order
    key = (core * tiles + lt) * 4 + run
    order = np.lexsort((lidx, key))             # addr-sorted within run
    ksort = key[order]
    nk = NCORES * tiles * 4
    cnt = np.bincount(ksort, minlength=nk)
    starts = np.concatenate([[0], np.cumsum(cnt)[:-1]])
    rank = np.arange(E, dtype=np.int64) - starts[ksort]

    cnt4 = cnt.reshape(-1, 4)
    C0 = max(1, int(np.ceil(cnt4[:, 0:2].max() / 128)))
    C1 = max(1, int(np.ceil(cnt4[:, 2:4].max() / 128)))
    CG = 2 * C0 + 2 * C1
    N0 = 2 * C0 * 128
    N1 = 2 * C1 * 128
    run_off = np.array([0, C0 * 128, 2 * C0 * 128, (2 * C0 + C1) * 128])
    SLOTS_G = CG * 128
    gidx = ksort // 4
    slot = gidx * SLOTS_G + run_off[ksort % 4] + rank
    TOT = NCORES * tiles * SLOTS_G

    IDX = np.zeros(TOT, np.int16)
    VAL = np.zeros(TOT, np.float32)
    DD = np.zeros(TOT, np.float32)
    I32 = np.zeros(TOT, np.int32)
    IDX[slot] = lidx[order]
    VAL[slot] = np.asarray(vals, np.float32)[order]
    DD[slot] = d[order]
    I32[slot] = col[order]

    IDX4 = IDX.reshape(NCORES, tiles, CG, 128)
    i0 = (IDX4[:, :, : 2 * C0, :].reshape(NCORES, tiles, N0 // 16, 16)
          .transpose(0, 1, 3, 2))
    i1 = (IDX4[:, :, 2 * C0 :, :].reshape(NCORES, tiles, N1 // 16, 16)
          .transpose(0, 1, 3, 2))
    idx16 = np.concatenate([i0, i1], axis=3)           # [NC, tiles, 16, IC]
    idx128 = np.ascontiguousarray(np.tile(idx16, (1, 1, 8, 1)))

    D4 = DD.reshape(NCORES, tiles, CG, 128).transpose(0, 1, 3, 2)
    V4 = VAL.reshape(NCORES, tiles, CG, 128).transpose(0, 1, 3, 2)
    dval = np.ascontiguousarray(
        np.concatenate([D4, V4, -V4], axis=3), dtype=np.float32)
    idx32 = np.ascontiguousarray(
        I32.reshape(NCORES, tiles, CG, 128).transpose(0, 1, 3, 2))

    table16 = np.asarray(mEmbed, np.float32).astype(np.float16)
    iota = np.ascontiguousarray(
        np.broadcast_to(np.arange(128, dtype=np.float16), (128, 128)))
    a = 2.0 * np.float32(np.asarray(inter).reshape(-1)[0])
    b = np.float32(2.0) - a
    ab = np.ascontiguousarray(
        np.stack([np.full(128, a, np.float32), np.full(128, b, np.float32)], axis=1))
    return C0, C1, table16, iota, ab, idx128, dval, idx32


def _in_maps(pre):
    C0, C1, table16, iota, ab, idx128, dval, idx32 = pre
    return [
        {"table": table16, "iota": iota, "ab": ab,
         "idx": idx128[k], "dval": dval[k], "idx32": idx32[k]}
        for k in range(NCORES)
    ]


# ---------------------------------------------------------------------------
# v2: per-tile variable chunk counts (max over the 8 cores) + merged per-half
# gather calls. Cuts gather descriptors and compute instructions by the
# padding slack of the old global-max layout (~7%), and halves the SWDGE
# call count.
# ---------------------------------------------------------------------------

def preprocess_v2(vals, mEmbed, inter, row_idx, col_idx, tiles=TILES):
    E = row_idx.shape[0]
    col = col_idx.astype(np.int64) % MED
    rowl = row_idx.astype(np.int64)
    plane = rowl // MED
    prow = rowl % MED
    core = np.minimum(prow // RPC, NCORES - 1)
    lt = (prow - core * RPC) >> 7
    d = (prow & 127).astype(np.float32)
    half = (col >= HALF).astype(np.int64)
    lidx = (col - half * HALF).astype(np.int16)

    run = half * 2 + plane                      # A0,B0,A1,B1 order
    key = (core * tiles + lt) * 4 + run
    order = np.lexsort((lidx, key))             # addr-sorted within run
    ksort = key[order]
    nk = NCORES * tiles * 4
    cnt = np.bincount(ksort, minlength=nk)
    starts = np.concatenate([[0], np.cumsum(cnt)[:-1]])
    rank = np.arange(E, dtype=np.int64) - starts[ksort]

    cnt3 = cnt.reshape(NCORES, tiles, 4)
    CT = np.maximum(1, -(-cnt3.max(axis=0) // 128))      # [tiles, 4]
    CG_t = CT.sum(axis=1)                                # [tiles]
    coff = np.concatenate(
        [np.zeros((tiles, 1), np.int64), np.cumsum(CT, axis=1)[:, :3]], axis=1)
    tile_base = np.concatenate([[0], np.cumsum(128 * CG_t)[:-1]])
    TOTC = int(128 * CG_t.sum())                         # slots per core

    lts = ksort // 4 % tiles
    runs = ksort % 4
    cores_s = ksort // (4 * tiles)
    slot = (cores_s * TOTC + tile_base[lts] + coff[lts, runs] * 128 + rank)

    IDX = np.zeros(NCORES * TOTC, np.int16)
    VAL = np.zeros(NCORES * TOTC, np.float32)
    DD = np.zeros(NCORES * TOTC, np.float32)
    IDX[slot] = lidx[order]
    VAL[slot] = np.asarray(vals, np.float32)[order]
    DD[slot] = d[order]
    IDX = IDX.reshape(NCORES, TOTC)
    VAL = VAL.reshape(NCORES, TOTC)
    DD = DD.reshape(NCORES, TOTC)

    # idx16 packing: per tile [16, IC_t] = [half0 | half1], each half wrapped
    # (N/16, 16)->T; tiled to 128 partitions. dval per tile [128, 2*CG_t].
    idx_parts, dval_parts = [], []
    for t in range(tiles):
        b = int(tile_base[t])
        n0 = int((CT[t, 0] + CT[t, 1]) * 128)
        n1 = int((CT[t, 2] + CT[t, 3]) * 128)
        i0 = IDX[:, b : b + n0].reshape(NCORES, n0 // 16, 16).transpose(0, 2, 1)
        i1 = (IDX[:, b + n0 : b + n0 + n1]
              .reshape(NCORES, n1 // 16, 16).transpose(0, 2, 1))
        idx_parts.append(np.concatenate([i0, i1], axis=2))
        cg = int(CG_t[t])
        dt_ = DD[:, b : b + 128 * cg].reshape(NCORES, cg, 128).transpose(0, 2, 1)
        vt_ = VAL[:, b : b + 128 * cg].reshape(NCORES, cg, 128).transpose(0, 2, 1)
        dval_parts.append(np.concatenate([dt_, vt_], axis=2))
    idx16 = np.concatenate(idx_parts, axis=2)            # [NC, 16, ICtot]
    idx128 = np.ascontiguousarray(np.tile(idx16, (1, 8, 1)))
    dval = np.ascontiguousarray(
        np.concatenate(dval_parts, axis=2), dtype=np.float32)

    table16 = np.asarray(mEmbed, np.float32).astype(np.float16)
    iota = np.ascontiguousarray(
        np.broadcast_to(np.arange(128, dtype=np.float16), (128, 128)))
    a = 2.0 * np.float32(np.asarray(inter).reshape(-1)[0])
    b_ = np.float32(2.0) - a
    ab = np.ascontiguousarray(
        np.stack([np.full(128, a, np.float32), np.full(128, b_, np.float32)],
                 axis=1))
    return tuple(map(tuple, CT.tolist())), table16, iota, ab, idx128, dval


def build_nc_v2(CT, tiles=TILES, gbufs=5, repeat=1, psbufs=4, spbufs=16):
    """CT: per-tile (CA0, CB0, CA1, CB1) chunk counts. Four gather calls per
    group (one per half x plane run), queues 0-3 — keeps all queues busy
    even at shallow pipeline depth."""
    CT = [tuple(c) for c in CT]
    ICs = [((c[0] + c[1]) * 8 + (c[2] + c[3]) * 8) for c in CT]
    CGs = [sum(c) for c in CT]
    ICtot = sum(ICs)
    DVtot = 2 * sum(CGs)
    dt16 = mybir.dt.float16
    f32 = mybir.dt.float32

    nc = bacc.Bacc(None, target_bir_lowering=False, num_swdge_queues=4)
    table = nc.dram_tensor("table", [MED, F], dt16, kind="ExternalInput")
    idx_d = nc.dram_tensor("idx", [P, ICtot], mybir.dt.int16,
                           kind="ExternalInput")
    dval_d = nc.dram_tensor("dval", [P, DVtot], f32, kind="ExternalInput")
    ab_d = nc.dram_tensor("ab", [P, 2], f32, kind="ExternalInput")
    iota_d = nc.dram_tensor("iota", [P, P], dt16, kind="ExternalInput")
    out_d = nc.dram_tensor("out", [tiles, P, F], f32, kind="ExternalOutput")

    ic_off = np.concatenate([[0], np.cumsum(ICs)[:-1]])
    dv_off = np.concatenate([[0], np.cumsum([2 * c for c in CGs])[:-1]])

    with tile.TileContext(nc) as tc:
        with (
            tc.tile_pool(name="const", bufs=1) as constp,
            tc.tile_pool(name="gbuf", bufs=gbufs) as gbufp,
            tc.tile_pool(name="meta", bufs=4) as metap,
            tc.tile_pool(name="sp", bufs=spbufs) as sp,
            tc.tile_pool(name="ep", bufs=6) as ep,
            tc.tile_pool(name="psum", bufs=psbufs,
                         space=bass.MemorySpace.PSUM) as psp,
        ):
            iota_t = constp.tile([P, P], dt16, tag="iota")
            nc.sync.dma_start(iota_t[:], iota_d[:])
            ab_t = constp.tile([P, 2], f32, tag="ab")
            nc.sync.dma_start(ab_t[:], ab_d[:])

            for gi, g in enumerate(
                [g_ for _ in range(repeat) for g_ in range(tiles)]
            ):
                CA0, CB0, CA1, CB1 = CT[g]
                CG = CA0 + CB0 + CA1 + CB1
                CH0 = CA0 + CB0
                IC = ICs[g]
                io, dvo = int(ic_off[g]), int(dv_off[g])
                planes = [0] * CA0 + [1] * CB0 + [0] * CA1 + [1] * CB1
                firstA, lastA = 0, CH0 + CA1 - 1
                firstB, lastB = CA0, CG - 1

                idx_t = metap.tile([P, IC], mybir.dt.int16, tag="idx")
                nc.sync.dma_start(idx_t[:], idx_d[:, io : io + IC])
                dv_t = metap.tile([P, 2 * CG], f32, tag="dval")
                nc.sync.dma_start(dv_t[:], dval_d[:, dvo : dvo + 2 * CG])

                g_t = gbufp.tile([P, CG, F], dt16, tag="g")
                calls = [(0, CA0, 0), (CA0, CH0, 1),
                         (CH0, CH0 + CA1, 2), (CH0 + CA1, CG, 3)]
                for (clo, chi, q) in calls:
                    n = (chi - clo) * 128
                    tlo, thi = (0, HALF) if chi <= CH0 else (HALF, MED)
                    nc.gpsimd.dma_gather(
                        g_t[:, clo:chi, :], table[tlo:thi, :],
                        idx_t[:, 8 * clo : 8 * chi], n, n, F,
                        single_packet=False, queue_num=q,
                    )

                psA = psp.tile([P, F], f32, tag="psA")
                psB = psp.tile([P, F], f32, tag="psB")
                for c in range(CG):
                    s_t = sp.tile([P, P], dt16, tag="s")
                    nc.vector.tensor_scalar(
                        s_t[:], iota_t[:],
                        dv_t[:, c : c + 1], dv_t[:, CG + c : CG + c + 1],
                        mybir.AluOpType.is_equal, mybir.AluOpType.mult,
                    )
                    if planes[c] == 0:
                        nc.tensor.matmul(psA[:], s_t[:], g_t[:, c, :],
                                         start=(c == firstA), stop=(c == lastA))
                    else:
                        nc.tensor.matmul(psB[:], s_t[:], g_t[:, c, :],
                                         start=(c == firstB), stop=(c == lastB))

                t0 = ep.tile([P, F], f32, tag="t0")
                nc.vector.tensor_scalar(t0[:], psA[:], 0.0, ab_t[:, 0:1],
                                        mybir.AluOpType.max, mybir.AluOpType.mult)
                t1 = ep.tile([P, F], f32, tag="t1")
                nc.vector.tensor_scalar(t1[:], psB[:], 0.0, ab_t[:, 1:2],
                                        mybir.AluOpType.max, mybir.AluOpType.mult)
                o_t = ep.tile([P, F], f32, tag="o")
                nc.vector.tensor_tensor(o_t[:], t0[:], t1[:], mybir.AluOpType.add)
                nc.sync.dma_start(out_d[g], o_t[:])

    nc.compile()
    return nc


def _in_maps_v2(pre):
    CT, table16, iota, ab, idx128, dval = pre
    return [
        {"table": table16, "iota": iota, "ab": ab,
         "idx": idx128[k], "dval": dval[k]}
        for k in range(NCORES)
    ]


def _run_v2(vals, mEmbed, inter, row_idx, col_idx, trace=False,
            build_kwargs=None):
    pre = preprocess_v2(vals, mEmbed, inter, row_idx, col_idx)
    CT = pre[0]
    bk = dict(build_kwargs or {})
    key = ("v2", CT, 1, tuple(sorted(bk.items())))
    if key not in _NC_CACHE:
        _NC_CACHE[key] = build_nc_v2(CT, **bk)
    nc = _NC_CACHE[key]
    res = run_bass_kernel_spmd(nc, _in_maps_v2(pre), core_ids=list(range(NCORES)),
                               trace=trace)
    full = np.concatenate(
        [res.results[k]["out"].reshape(RPC, F) for k in range(NCORES)], axis=0)
    return np.ascontiguousarray(full[:MED]), res


def _run(vals, mEmbed, inter, row_idx, col_idx, trace=False, build_kwargs=None):
    pre = preprocess(vals, mEmbed, inter, row_idx, col_idx)
    C0, C1 = pre[0], pre[1]
    bk = dict(build_kwargs or {})
    key = (C0, C1, 1, tuple(sorted(bk.items())))
    if key not in _NC_CACHE:
        _NC_CACHE[key] = build_nc(C0, C1, **bk)
    nc = _NC_CACHE[key]
    res = run_bass_kernel_spmd(nc, _in_maps(pre), core_ids=list(range(NCORES)),
                               trace=trace)
    full = np.concatenate(
        [res.results[k]["out"].reshape(RPC, F) for k in range(NCORES)], axis=0)
    return np.ascontiguousarray(full[:MED]), res


def kernel(vals, mEmbed, inter, row_idx, col_idx):
    out, _ = _run(vals, mEmbed, inter, row_idx, col_idx, trace=False)
    return out


def _make_sharded(nc, donate=False):
    """Replicate bass2jax.run_bass_via_pjrt's executable construction so we
    can reuse it for repeated timed executions."""
    import jax
    from jax.sharding import Mesh, PartitionSpec
    from jax.experimental.shard_map import shard_map
    from concourse import bass2jax as b2j

    b2j.install_neuronx_cc_hook()
    partition_name = nc.partition_id_tensor.name if nc.partition_id_tensor else None
    in_names, out_names, out_avals, zero_outs = [], [], [], []
    for alloc in nc.m.functions[0].allocations:
        if not isinstance(alloc, mybir.MemoryLocationSet):
            continue
        name = alloc.memorylocations[0].name
        if alloc.kind == "ExternalInput":
            if name != partition_name:
                in_names.append(name)
        elif alloc.kind == "ExternalOutput":
            out_names.append(name)
            shape = tuple(alloc.tensor_shape)
            dtype = mybir.dt.np(alloc.dtype)
            out_avals.append(jax.core.ShapedArray(shape, dtype))
            zero_outs.append(np.zeros(shape, dtype))
    n_params = len(in_names)
    in_names = in_names + out_names
    if partition_name is not None:
        in_names = in_names + [partition_name]

    def _body(*args):
        operands = list(args)
        if partition_name is not None:
            operands.append(b2j.partition_id_tensor())
        outs = b2j._bass_exec_p.bind(
            *operands,
            out_avals=tuple(out_avals),
            in_names=tuple(in_names),
            out_names=tuple(out_names),
            lowering_input_output_aliases=(),
            sim_require_finite=True,
            sim_require_nnan=True,
            nc=nc,
        )
        return tuple(outs)

    devices = jax.devices()[:NCORES]
    mesh = Mesh(np.asarray(devices), ("core",))
    in_specs = (PartitionSpec("core"),) * (n_params + len(out_names))
    out_specs = (PartitionSpec("core"),) * len(out_names)
    kw = dict(donate_argnums=tuple(range(n_params, n_params + len(out_names)))) if donate else {}

    sharded = jax.jit(
        shard_map(_body, mesh=mesh, in_specs=in_specs,
                  out_specs=out_specs, check_rep=False),
        keep_unused=True, **kw)
    return sharded, mesh, in_names[:n_params], out_names, zero_outs


def timed_run(vals, mEmbed, inter, row_idx, col_idx, k=9, samples=12,
              build_kwargs=None, use_v2=False):
    """Time on device: build the same program with the body repeated 1x and
    kx INSIDE the NEFF; marginal = (median T(k) - median T(1)) / (k-1) =
    pure HW time (per-call dispatch overhead and tunnel latency cancel)."""
    import time
    import jax
    from jax.sharding import NamedSharding, PartitionSpec

    bk = dict(build_kwargs or {})
    if use_v2:
        pre = preprocess_v2(vals, mEmbed, inter, row_idx, col_idx)
        ckey = pre[0]
        per_core = _in_maps_v2(pre)
        builder = lambda repeat: build_nc_v2(ckey, repeat=repeat, **bk)
    else:
        pre = preprocess(vals, mEmbed, inter, row_idx, col_idx)
        C0, C1 = pre[0], pre[1]
        ckey = (C0, C1)
        per_core = _in_maps(pre)
        builder = lambda repeat: build_nc(C0, C1, repeat=repeat, **bk)

    def make_runner(repeat):
        ck = (use_v2, ckey, repeat, tuple(sorted(bk.items())))
        if ck not in _NC_CACHE:
            _NC_CACHE[ck] = builder(repeat)
        nc = _NC_CACHE[ck]
        sharded, mesh, in_names, out_names, zero_outs = _make_sharded(nc)
        sh = NamedSharding(mesh, PartitionSpec("core"))
        shapes = {}
        for alloc in nc.m.functions[0].allocations:
            if isinstance(alloc, mybir.MemoryLocationSet) and alloc.kind == "ExternalInput":
                shapes[alloc.memorylocations[0].name] = (
                    tuple(alloc.tensor_shape), mybir.dt.np(alloc.dtype))
        def get_in(c, n):
            if n in per_core[c]:
                return np.asarray(per_core[c][n])
            shp, dt = shapes[n]
            return np.zeros(shp, dt)
        concat_in = [
            jax.device_put(
                np.concatenate([get_in(c, n) for c in range(NCORES)], axis=0), sh)
            for n in in_names
        ]
        concat_zero = [
            jax.device_put(np.zeros((NCORES * z.shape[0], *z.shape[1:]), z.dtype), sh)
            for z in zero_outs
        ]
        def run():
            out = sharded(*concat_in, *concat_zero)
            jax.block_until_ready(out)
        run()   # warm-up / compile
        return run

    r1 = make_runner(1)
    rk = make_runner(k)

    # Model switches cost ~10 ms through the tunnel, so time each NEFF in
    # bursts of consecutive calls (drop the first two after each switch) and
    # alternate bursts to cancel slow drift in the per-call overhead.
    def burst(run, n=6, discard=2):
        ts = []
        for _ in range(n):
            t0 = time.perf_counter(); run(); ts.append(time.perf_counter() - t0)
        return ts[discard:]

    t1s, tks = [], []
    for _ in range(max(6, samples // 2)):
        t1s += burst(r1)
        tks += burst(rk)
    t1 = float(np.median(t1s))
    tk = float(np.median(tks))
    marginal_ns = (tk - t1) / (k - 1) * 1e9
    return int(marginal_ns), int(t1 * 1e9), int(tk * 1e9)



# revision 38
# speedup vs baseline: 1.2666x; 1.0057x over previous
"""Trainium2 Bass kernel: GCN message passing (nn_DDI_gcn), 8 NeuronCores SPMD.

Math:
  agg[r] = sum_{e: row_idx[e]==r} vals[e] * mEmbed[col_idx[e] % 50000]
  out[i] = 2*(inter*relu(agg[i]) + (1-inter)*relu(agg[i+50000])),  i < 50000

Strategy (destination sharding, no cross-core reduction):
  * Core k owns output rows [6272k, 6272(k+1)); host buckets every edge by
    (core, 128-row dest tile, table half, plane) and pads each bucket run to
    a 128-edge chunk boundary. Edges are sorted by gather address within each
    run (HBM locality) and pad slots carry index -1.
  * Device, per dest tile ("group"): four+ dma_gathers (flat chunk list
    split evenly over 4 SWDGE queues for parallel descriptor processing)
    fetch the edges' embedding rows (fp16, 256 B/row) into SBUF. For each
    128-edge chunk build the selection matrix S[e, r] = val[e]*(d[e]==r) with
    ONE dual-op tensor_scalar (is_equal -> mult) from a constant iota tile;
    TensorE accumulates S^T @ G into a per-plane PSUM tile (the segment sum);
    epilogue applies a*relu(psumA) + b*relu(psumB) and streams the 128x128
    f32 tile out.
  * All index math is host-side numpy; the device never touches raw indices
    except as dma_gather int16 offsets.
"""

import numpy as np

import concourse.bass as bass
import concourse.bacc as bacc
import concourse.tile as tile
import concourse.mybir as mybir
from concourse.bass_utils import run_bass_kernel_spmd

AF = mybir.ActivationFunctionType

MED = 50000
NCORES = 8
TILES = 49               # dest tiles per plane per core
RPC = TILES * 128        # 6272 dest rows per core (per plane)
HALF = 25000             # equal table halves: balanced SWDGE queue loads
                         # (any split <= 32768 keeps gather idxs in int16)
P = 128
F = 128                  # feature dim

_NC_CACHE = {}


def build_nc(C0, C1, tiles=TILES, gbufs=7, repeat=1,
             do_gather=True, do_compute=True, probe=None, scratch=16384,
             cstride=1, actmod=0, sp1=False, dvespam=0, merge_calls=False):
    """C0/C1: chunks per half0/half1 run. Group chunk layout: [A0|B0|A1|B1].
    repeat>1 re-runs the whole body (timing: marginal = pure HW time).
    do_gather/do_compute: microbenchmark switches (timing only).
    probe: None | 'seqdma' (replace gather w/ sequential DMA of same volume)
         | 'e2x' (gather 512B descs, same desc count, 2x bytes)
         | 'q1'  (all gathers on one SWDGE queue).
    cstride: compute only every cstride-th chunk (timing probe)."""
    CG = 2 * C0 + 2 * C1
    N0 = 2 * C0 * 128    # idxs in table half 0
    N1 = 2 * C1 * 128
    IC = (N0 + N1) // 16
    dt16 = mybir.dt.float16
    f32 = mybir.dt.float32

    nc = bacc.Bacc(None, target_bir_lowering=False, num_swdge_queues=4,
                   dynamic_dma_scratch_size=scratch)
    table = nc.dram_tensor("table", [MED, F], dt16, kind="ExternalInput")
    # dval layout: [d | v | -v] (3*CG columns); -v feeds the Act-engine
    # S-build (scale=-v, bias=+v).
    idx32_d = (nc.dram_tensor("idx32", [tiles, P, CG], mybir.dt.int32,
                              kind="ExternalInput")
               if probe in ("ind", "hybrid") else None)
    t2x = (nc.dram_tensor("t2x", [HALF, 2 * F], dt16, kind="ExternalInput")
           if probe == "e2x" else None)
    tbig = (nc.dram_tensor("tbig", [P, CG, F], dt16, kind="ExternalInput")
            if probe == "seqdma" else None)
    idx_d = nc.dram_tensor("idx", [tiles, P, IC], mybir.dt.int16, kind="ExternalInput")
    dval_d = nc.dram_tensor("dval", [tiles, P, 3 * CG], f32, kind="ExternalInput")
    ab_d = nc.dram_tensor("ab", [P, 2], f32, kind="ExternalInput")
    iota_d = nc.dram_tensor("iota", [P, P], dt16, kind="ExternalInput")
    out_d = nc.dram_tensor("out", [tiles, P, F], f32, kind="ExternalOutput")

    planes = [0] * C0 + [1] * C0 + [0] * C1 + [1] * C1
    firstA, lastA = 0, 2 * C0 + C1 - 1
    firstB, lastB = C0, CG - 1

    # One gather call per run (A0,B0,A1,B1), one SWDGE queue each. The call
    # pattern must be 4-periodic so the round-robin DMASW sem lanes (8) stay
    # consistently locked to their queues.
    # (chunk_lo, chunk_hi, queue)
    calls = [
        (0, C0, 0),
        (C0, 2 * C0, 1),
        (2 * C0, 2 * C0 + C1, 2),
        (2 * C0 + C1, CG, 3),
    ]

    def idx_cols(c):  # idx column of chunk c's first index
        return 8 * c if c <= 2 * C0 else N0 // 16 + 8 * (c - 2 * C0)

    with tile.TileContext(nc) as tc:
        with (
            tc.tile_pool(name="const", bufs=1) as constp,
            tc.tile_pool(name="gbuf", bufs=gbufs) as gbufp,
            tc.tile_pool(name="meta", bufs=4) as metap,
            tc.tile_pool(name="sp", bufs=16) as sp,
            tc.tile_pool(name="ep", bufs=6) as ep,
            tc.tile_pool(name="psum", bufs=4, space=bass.MemorySpace.PSUM) as psp,
        ):
            iota_t = constp.tile([P, P], dt16, tag="iota")
            nc.sync.dma_start(iota_t[:], iota_d[:])
            ab_t = constp.tile([P, 2], f32, tag="ab")
            nc.sync.dma_start(ab_t[:], ab_d[:])

            for gi, g in enumerate(
                [g_ for _ in range(repeat) for g_ in range(tiles)]
            ):
                idx_t = metap.tile([P, IC], mybir.dt.int16, tag="idx")
                nc.sync.dma_start(idx_t[:], idx_d[g])
                dv_t = metap.tile([P, 3 * CG], f32, tag="dval")
                nc.sync.dma_start(dv_t[:], dval_d[g])

                gw = 2 * F if probe == "e2x" else F
                g_t = gbufp.tile([P, CG, gw], dt16, tag="g")
                if probe in ("ind", "hybrid"):
                    i32_t = metap.tile([P, CG], mybir.dt.int32, tag="i32")
                    nc.sync.dma_start(i32_t[:], idx32_d[g])
                if probe == "seqdma":
                    nc.sync.dma_start(g_t[:], tbig[:])
                elif probe == "ind":
                    nc.gpsimd.indirect_dma_start(
                        g_t[:, :, :], None, table[:, :],
                        bass.IndirectOffsetOnAxis(ap=i32_t[:, :], axis=0),
                    )
                elif probe == "hybrid":
                    # runs A0,B0 via SWDGE queues 0-1; A1,B1 via indirect
                    for (clo, chi, q) in calls[:2]:
                        n = (chi - clo) * 128
                        nc.gpsimd.dma_gather(
                            g_t[:, clo:chi, :], table[0:HALF, :],
                            idx_t[:, idx_cols(clo) : idx_cols(chi)], n, n, F,
                            single_packet=sp1, queue_num=q,
                        )
                    nc.gpsimd.indirect_dma_start(
                        g_t[:, 2 * C0 :, :], None, table[:, :],
                        bass.IndirectOffsetOnAxis(ap=i32_t[:, 2 * C0 :], axis=0),
                    )
                elif do_gather and merge_calls:
                    # one call per table half; queues alternate by group
                    for j, (clo, chi) in enumerate([(0, 2 * C0), (2 * C0, CG)]):
                        n = (chi - clo) * 128
                        tlo, thi = (0, HALF) if j == 0 else (HALF, MED)
                        nc.gpsimd.dma_gather(
                            g_t[:, clo:chi, :], table[tlo:thi, :],
                            idx_t[:, idx_cols(clo) : idx_cols(chi)], n, n, F,
                            single_packet=False,
                            queue_num=2 * (g % 2) + j,
                        )
                elif do_gather:
                    for (clo, chi, q) in calls:
                        n = (chi - clo) * 128
                        if probe == "e2x":
                            nc.gpsimd.dma_gather(
                                g_t[:, clo:chi, :], t2x[:, :],
                                idx_t[:, idx_cols(clo) : idx_cols(chi)], n, n,
                                2 * F, single_packet=False, queue_num=q,
                            )
                            continue
                        tlo, thi = (0, HALF) if chi <= 2 * C0 else (HALF, MED)
                        nc.gpsimd.dma_gather(
                            g_t[:, clo:chi, :], table[tlo:thi, :],
                            idx_t[:, idx_cols(clo) : idx_cols(chi)], n, n, F,
                            single_packet=sp1,
                            queue_num=0 if probe == "q1" else q,
                        )
                for _ in range(dvespam):
                    sp_t = sp.tile([P, P], dt16, tag="spam")
                    nc.vector.tensor_scalar(
                        sp_t[:], iota_t[:], 1.0, 2.0,
                        mybir.AluOpType.mult, mybir.AluOpType.add)
                if not do_compute:
                    continue

                psA = psp.tile([P, F], f32, tag="psA")
                psB = psp.tile([P, F], f32, tag="psB")
                kept = [c for c in range(CG) if c % cstride == 0]
                kA = [c for c in kept if planes[c] == 0]
                kB = [c for c in kept if planes[c] == 1]
                for ci, c in enumerate(kept):
                    s_t = sp.tile([P, P], dt16, tag="s")
                    if actmod and ci % actmod == actmod - 1:
                        # Act-engine S-build: |j-d| then relu(v - v*|j-d|)
                        a1 = sp.tile([P, P], dt16, tag="a1")
                        nc.scalar.activation(
                            a1[:], iota_t[:], AF.Abs,
                            bias=dv_t[:, c : c + 1], scale=-1.0)
                        nc.scalar.activation(
                            s_t[:], a1[:], AF.Relu,
                            bias=dv_t[:, CG + c : CG + c + 1],
                            scale=dv_t[:, 2 * CG + c : 2 * CG + c + 1])
                    else:
                        nc.vector.tensor_scalar(
                            s_t[:], iota_t[:],
                            dv_t[:, c : c + 1], dv_t[:, CG + c : CG + c + 1],
                            mybir.AluOpType.is_equal, mybir.AluOpType.mult,
                        )
                    if planes[c] == 0:
                        nc.tensor.matmul(psA[:], s_t[:], g_t[:, c, :],
                                         start=(c == kA[0]), stop=(c == kA[-1]))
                    else:
                        nc.tensor.matmul(psB[:], s_t[:], g_t[:, c, :],
                                         start=(c == kB[0]), stop=(c == kB[-1]))

                t0 = ep.tile([P, F], f32, tag="t0")
                nc.vector.tensor_scalar(t0[:], psA[:], 0.0, ab_t[:, 0:1],
                                        mybir.AluOpType.max, mybir.AluOpType.mult)
                t1 = ep.tile([P, F], f32, tag="t1")
                nc.vector.tensor_scalar(t1[:], psB[:], 0.0, ab_t[:, 1:2],
                                        mybir.AluOpType.max, mybir.AluOpType.mult)
                o_t = ep.tile([P, F], f32, tag="o")
                nc.vector.tensor_tensor(o_t[:], t0[:], t1[:], mybir.AluOpType.add)
                nc.sync.dma_start(out_d[g], o_t[:])

    nc.compile()
    return nc


def preprocess(vals, mEmbed, inter, row_idx, col_idx, tiles=TILES):
    E = row_idx.shape[0]
    col = col_idx.astype(np.int64) % MED
    rowl = row_idx.astype(np.int64)
    plane = rowl // MED
    prow = rowl % MED
    core = np.minimum(prow // RPC, NCORES - 1)
    lt = (prow - core * RPC) >> 7
    d = (prow & 127).astype(np.float32)
    half = (col >= HALF).astype(np.int64)
    lidx = (col - half * HALF).astype(np.int16)

    run = half * 2 + plane                      # A0,B0,A1,B1 order
    key = (core * tiles + lt) * 4 + run
    order = np.lexsort((lidx, key))             # addr-sorted within run
    ksort = key[order]
    nk = NCORES * tiles * 4
    cnt = np.bincount(ksort, minlength=nk)
    starts = np.concatenate([[0], np.cumsum(cnt)[:-1]])
    rank = np.arange(E, dtype=np.int64) - starts[ksort]

    cnt4 = cnt.reshape(-1, 4)
    C0 = max(1, int(np.ceil(cnt4[:, 0:2].max() / 128)))
    C1 = max(1, int(np.ceil(cnt4[:, 2:4].max() / 128)))
    CG = 2 * C0 + 2 * C1
    N0 = 2 * C0 * 128
    N1 = 2 * C1 * 128
    run_off = np.array([0, C0 * 128, 2 * C0 * 128, (2 * C0 + C1) * 128])
    SLOTS_G = CG * 128
    gidx = ksort // 4
    slot = gidx * SLOTS_G + run_off[ksort % 4] + rank
    TOT = NCORES * tiles * SLOTS_G

    IDX = np.zeros(TOT, np.int16)
    VAL = np.zeros(TOT, np.float32)
    DD = np.zeros(TOT, np.float32)
    I32 = np.zeros(TOT, np.int32)
    IDX[slot] = lidx[order]
    VAL[slot] = np.asarray(vals, np.float32)[order]
    DD[slot] = d[order]
    I32[slot] = col[order]

    IDX4 = IDX.reshape(NCORES, tiles, CG, 128)
    i0 = (IDX4[:, :, : 2 * C0, :].reshape(NCORES, tiles, N0 // 16, 16)
          .transpose(0, 1, 3, 2))
    i1 = (IDX4[:, :, 2 * C0 :, :].reshape(NCORES, tiles, N1 // 16, 16)
          .transpose(0, 1, 3, 2))
    idx16 = np.concatenate([i0, i1], axis=3)           # [NC, tiles, 16, IC]
    idx128 = np.ascontiguousarray(np.tile(idx16, (1, 1, 8, 1)))

    D4 = DD.reshape(NCORES, tiles, CG, 128).transpose(0, 1, 3, 2)
    V4 = VAL.reshape(NCORES, tiles, CG, 128).transpose(0, 1, 3, 2)
    dval = np.ascontiguousarray(
        np.concatenate([D4, V4, -V4], axis=3), dtype=np.float32)
    idx32 = np.ascontiguousarray(
        I32.reshape(NCORES, tiles, CG, 128).transpose(0, 1, 3, 2))

    table16 = np.asarray(mEmbed, np.float32).astype(np.float16)
    iota = np.ascontiguousarray(
        np.broadcast_to(np.arange(128, dtype=np.float16), (128, 128)))
    a = 2.0 * np.float32(np.asarray(inter).reshape(-1)[0])
    b = np.float32(2.0) - a
    ab = np.ascontiguousarray(
        np.stack([np.full(128, a, np.float32), np.full(128, b, np.float32)], axis=1))
    return C0, C1, table16, iota, ab, idx128, dval, idx32


def _in_maps(pre):
    C0, C1, table16, iota, ab, idx128, dval, idx32 = pre
    return [
        {"table": table16, "iota": iota, "ab": ab,
         "idx": idx128[k], "dval": dval[k], "idx32": idx32[k]}
        for k in range(NCORES)
    ]


# ---------------------------------------------------------------------------
# v2: per-tile variable chunk counts (max over the 8 cores) + merged per-half
# gather calls. Cuts gather descriptors and compute instructions by the
# padding slack of the old global-max layout (~7%), and halves the SWDGE
# call count.
# ---------------------------------------------------------------------------

def preprocess_v2(vals, mEmbed, inter, row_idx, col_idx, tiles=TILES):
    E = row_idx.shape[0]
    col = col_idx.astype(np.int64) % MED
    rowl = row_idx.astype(np.int64)
    plane = rowl // MED
    prow = rowl % MED
    core = np.minimum(prow // RPC, NCORES - 1)
    lt = (prow - core * RPC) >> 7
    d = (prow & 127).astype(np.float32)
    half = (col >= HALF).astype(np.int64)
    lidx = (col - half * HALF).astype(np.int16)

    run = half * 2 + plane                      # A0,B0,A1,B1 order
    key = (core * tiles + lt) * 4 + run
    order = np.lexsort((lidx, key))             # addr-sorted within run
    ksort = key[order]
    nk = NCORES * tiles * 4
    cnt = np.bincount(ksort, minlength=nk)
    starts = np.concatenate([[0], np.cumsum(cnt)[:-1]])
    rank = np.arange(E, dtype=np.int64) - starts[ksort]

    cnt3 = cnt.reshape(NCORES, tiles, 4)
    CT = np.maximum(1, -(-cnt3.max(axis=0) // 128))      # [tiles, 4]
    CG_t = CT.sum(axis=1)                                # [tiles]
    coff = np.concatenate(
        [np.zeros((tiles, 1), np.int64), np.cumsum(CT, axis=1)[:, :3]], axis=1)
    tile_base = np.concatenate([[0], np.cumsum(128 * CG_t)[:-1]])
    TOTC = int(128 * CG_t.sum())                         # slots per core

    lts = ksort // 4 % tiles
    runs = ksort % 4
    cores_s = ksort // (4 * tiles)
    slot = (cores_s * TOTC + tile_base[lts] + coff[lts, runs] * 128 + rank)

    IDX = np.zeros(NCORES * TOTC, np.int16)
    VAL = np.zeros(NCORES * TOTC, np.float32)
    DD = np.zeros(NCORES * TOTC, np.float32)
    IDX[slot] = lidx[order]
    VAL[slot] = np.asarray(vals, np.float32)[order]
    DD[slot] = d[order]
    IDX = IDX.reshape(NCORES, TOTC)
    VAL = VAL.reshape(NCORES, TOTC)
    DD = DD.reshape(NCORES, TOTC)

    # idx16 packing: per tile [16, IC_t] = [half0 | half1], each half wrapped
    # (N/16, 16)->T; tiled to 128 partitions. dval per tile [128, 2*CG_t].
    idx_parts, dval_parts = [], []
    for t in range(tiles):
        b = int(tile_base[t])
        n0 = int((CT[t, 0] + CT[t, 1]) * 128)
        n1 = int((CT[t, 2] + CT[t, 3]) * 128)
        i0 = IDX[:, b : b + n0].reshape(NCORES, n0 // 16, 16).transpose(0, 2, 1)
        i1 = (IDX[:, b + n0 : b + n0 + n1]
              .reshape(NCORES, n1 // 16, 16).transpose(0, 2, 1))
        idx_parts.append(np.concatenate([i0, i1], axis=2))
        cg = int(CG_t[t])
        dt_ = DD[:, b : b + 128 * cg].reshape(NCORES, cg, 128).transpose(0, 2, 1)
        vt_ = VAL[:, b : b + 128 * cg].reshape(NCORES, cg, 128).transpose(0, 2, 1)
        dval_parts.append(np.concatenate([dt_, vt_], axis=2))
    idx16 = np.concatenate(idx_parts, axis=2)            # [NC, 16, ICtot]
    idx128 = np.ascontiguousarray(np.tile(idx16, (1, 8, 1)))
    dval = np.ascontiguousarray(
        np.concatenate(dval_parts, axis=2), dtype=np.float32)

    table16 = np.asarray(mEmbed, np.float32).astype(np.float16)
    iota = np.ascontiguousarray(
        np.broadcast_to(np.arange(128, dtype=np.float16), (128, 128)))
    a = 2.0 * np.float32(np.asarray(inter).reshape(-1)[0])
    b_ = np.float32(2.0) - a
    ab = np.ascontiguousarray(
        np.stack([np.full(128, a, np.float32), np.full(128, b_, np.float32)],
                 axis=1))
    return tuple(map(tuple, CT.tolist())), table16, iota, ab, idx128, dval


def build_nc_v2(CT, tiles=TILES, gbufs=7, repeat=1, psbufs=4, spbufs=16):
    """CT: per-tile (CA0, CB0, CA1, CB1) chunk counts. Four gather calls per
    group (one per half x plane run), queues 0-3 — keeps all queues busy
    even at shallow pipeline depth."""
    CT = [tuple(c) for c in CT]
    ICs = [((c[0] + c[1]) * 8 + (c[2] + c[3]) * 8) for c in CT]
    CGs = [sum(c) for c in CT]
    ICtot = sum(ICs)
    DVtot = 2 * sum(CGs)
    dt16 = mybir.dt.float16
    f32 = mybir.dt.float32

    nc = bacc.Bacc(None, target_bir_lowering=False, num_swdge_queues=4)
    table = nc.dram_tensor("table", [MED, F], dt16, kind="ExternalInput")
    idx_d = nc.dram_tensor("idx", [P, ICtot], mybir.dt.int16,
                           kind="ExternalInput")
    dval_d = nc.dram_tensor("dval", [P, DVtot], f32, kind="ExternalInput")
    ab_d = nc.dram_tensor("ab", [P, 2], f32, kind="ExternalInput")
    iota_d = nc.dram_tensor("iota", [P, P], dt16, kind="ExternalInput")
    out_d = nc.dram_tensor("out", [tiles, P, F], f32, kind="ExternalOutput")

    ic_off = np.concatenate([[0], np.cumsum(ICs)[:-1]])
    dv_off = np.concatenate([[0], np.cumsum([2 * c for c in CGs])[:-1]])

    with tile.TileContext(nc) as tc:
        with (
            tc.tile_pool(name="const", bufs=1) as constp,
            tc.tile_pool(name="gbuf", bufs=gbufs) as gbufp,
            tc.tile_pool(name="meta", bufs=4) as metap,
            tc.tile_pool(name="sp", bufs=spbufs) as sp,
            tc.tile_pool(name="ep", bufs=6) as ep,
            tc.tile_pool(name="psum", bufs=psbufs,
                         space=bass.MemorySpace.PSUM) as psp,
        ):
            iota_t = constp.tile([P, P], dt16, tag="iota")
            nc.sync.dma_start(iota_t[:], iota_d[:])
            ab_t = constp.tile([P, 2], f32, tag="ab")
            nc.sync.dma_start(ab_t[:], ab_d[:])

            for gi, g in enumerate(
                [g_ for _ in range(repeat) for g_ in range(tiles)]
            ):
                CA0, CB0, CA1, CB1 = CT[g]
                CG = CA0 + CB0 + CA1 + CB1
                CH0 = CA0 + CB0
                IC = ICs[g]
                io, dvo = int(ic_off[g]), int(dv_off[g])
                planes = [0] * CA0 + [1] * CB0 + [0] * CA1 + [1] * CB1
                firstA, lastA = 0, CH0 + CA1 - 1
                firstB, lastB = CA0, CG - 1

                idx_t = metap.tile([P, IC], mybir.dt.int16, tag="idx")
                nc.sync.dma_start(idx_t[:], idx_d[:, io : io + IC])
                dv_t = metap.tile([P, 2 * CG], f32, tag="dval")
                nc.sync.dma_start(dv_t[:], dval_d[:, dvo : dvo + 2 * CG])

                g_t = gbufp.tile([P, CG, F], dt16, tag="g")
                calls = [(0, CA0, 0), (CA0, CH0, 1),
                         (CH0, CH0 + CA1, 2), (CH0 + CA1, CG, 3)]
                for (clo, chi, q) in calls:
                    n = (chi - clo) * 128
                    tlo, thi = (0, HALF) if chi <= CH0 else (HALF, MED)
                    nc.gpsimd.dma_gather(
                        g_t[:, clo:chi, :], table[tlo:thi, :],
                        idx_t[:, 8 * clo : 8 * chi], n, n, F,
                        single_packet=False, queue_num=q,
                    )

                psA = psp.tile([P, F], f32, tag="psA")
                psB = psp.tile([P, F], f32, tag="psB")
                for c in range(CG):
                    s_t = sp.tile([P, P], dt16, tag="s")
                    nc.vector.tensor_scalar(
                        s_t[:], iota_t[:],
                        dv_t[:, c : c + 1], dv_t[:, CG + c : CG + c + 1],
                        mybir.AluOpType.is_equal, mybir.AluOpType.mult,
                    )
                    if planes[c] == 0:
                        nc.tensor.matmul(psA[:], s_t[:], g_t[:, c, :],
                                         start=(c == firstA), stop=(c == lastA))
                    else:
                        nc.tensor.matmul(psB[:], s_t[:], g_t[:, c, :],
                                         start=(c == firstB), stop=(c == lastB))

                t0 = ep.tile([P, F], f32, tag="t0")
                nc.vector.tensor_scalar(t0[:], psA[:], 0.0, ab_t[:, 0:1],
                                        mybir.AluOpType.max, mybir.AluOpType.mult)
                t1 = ep.tile([P, F], f32, tag="t1")
                nc.vector.tensor_scalar(t1[:], psB[:], 0.0, ab_t[:, 1:2],
                                        mybir.AluOpType.max, mybir.AluOpType.mult)
                o_t = ep.tile([P, F], f32, tag="o")
                nc.vector.tensor_tensor(o_t[:], t0[:], t1[:], mybir.AluOpType.add)
                nc.sync.dma_start(out_d[g], o_t[:])

    nc.compile()
    return nc


def _in_maps_v2(pre):
    CT, table16, iota, ab, idx128, dval = pre
    return [
        {"table": table16, "iota": iota, "ab": ab,
         "idx": idx128[k], "dval": dval[k]}
        for k in range(NCORES)
    ]


def _run_v2(vals, mEmbed, inter, row_idx, col_idx, trace=False,
            build_kwargs=None):
    pre = preprocess_v2(vals, mEmbed, inter, row_idx, col_idx)
    CT = pre[0]
    bk = dict(build_kwargs or {})
    key = ("v2", CT, 1, tuple(sorted(bk.items())))
    if key not in _NC_CACHE:
        _NC_CACHE[key] = build_nc_v2(CT, **bk)
    nc = _NC_CACHE[key]
    res = run_bass_kernel_spmd(nc, _in_maps_v2(pre), core_ids=list(range(NCORES)),
                               trace=trace)
    full = np.concatenate(
        [res.results[k]["out"].reshape(RPC, F) for k in range(NCORES)], axis=0)
    return np.ascontiguousarray(full[:MED]), res


def _run(vals, mEmbed, inter, row_idx, col_idx, trace=False, build_kwargs=None):
    pre = preprocess(vals, mEmbed, inter, row_idx, col_idx)
    C0, C1 = pre[0], pre[1]
    bk = dict(build_kwargs or {})
    key = (C0, C1, 1, tuple(sorted(bk.items())))
    if key not in _NC_CACHE:
        _NC_CACHE[key] = build_nc(C0, C1, **bk)
    nc = _NC_CACHE[key]
    res = run_bass_kernel_spmd(nc, _in_maps(pre), core_ids=list(range(NCORES)),
                               trace=trace)
    full = np.concatenate(
        [res.results[k]["out"].reshape(RPC, F) for k in range(NCORES)], axis=0)
    return np.ascontiguousarray(full[:MED]), res


def kernel(vals, mEmbed, inter, row_idx, col_idx):
    out, _ = _run_v2(vals, mEmbed, inter, row_idx, col_idx, trace=False)
    return out


def _make_sharded(nc, donate=False):
    """Replicate bass2jax.run_bass_via_pjrt's executable construction so we
    can reuse it for repeated timed executions."""
    import jax
    from jax.sharding import Mesh, PartitionSpec
    from jax.experimental.shard_map import shard_map
    from concourse import bass2jax as b2j

    b2j.install_neuronx_cc_hook()
    partition_name = nc.partition_id_tensor.name if nc.partition_id_tensor else None
    in_names, out_names, out_avals, zero_outs = [], [], [], []
    for alloc in nc.m.functions[0].allocations:
        if not isinstance(alloc, mybir.MemoryLocationSet):
            continue
        name = alloc.memorylocations[0].name
        if alloc.kind == "ExternalInput":
            if name != partition_name:
                in_names.append(name)
        elif alloc.kind == "ExternalOutput":
            out_names.append(name)
            shape = tuple(alloc.tensor_shape)
            dtype = mybir.dt.np(alloc.dtype)
            out_avals.append(jax.core.ShapedArray(shape, dtype))
            zero_outs.append(np.zeros(shape, dtype))
    n_params = len(in_names)
    in_names = in_names + out_names
    if partition_name is not None:
        in_names = in_names + [partition_name]

    def _body(*args):
        operands = list(args)
        if partition_name is not None:
            operands.append(b2j.partition_id_tensor())
        outs = b2j._bass_exec_p.bind(
            *operands,
            out_avals=tuple(out_avals),
            in_names=tuple(in_names),
            out_names=tuple(out_names),
            lowering_input_output_aliases=(),
            sim_require_finite=True,
            sim_require_nnan=True,
            nc=nc,
        )
        return tuple(outs)

    devices = jax.devices()[:NCORES]
    mesh = Mesh(np.asarray(devices), ("core",))
    in_specs = (PartitionSpec("core"),) * (n_params + len(out_names))
    out_specs = (PartitionSpec("core"),) * len(out_names)
    kw = dict(donate_argnums=tuple(range(n_params, n_params + len(out_names)))) if donate else {}

    sharded = jax.jit(
        shard_map(_body, mesh=mesh, in_specs=in_specs,
                  out_specs=out_specs, check_rep=False),
        keep_unused=True, **kw)
    return sharded, mesh, in_names[:n_params], out_names, zero_outs


def timed_run(vals, mEmbed, inter, row_idx, col_idx, k=9, samples=12,
              build_kwargs=None, use_v2=True):
    """Time on device: build the same program with the body repeated 1x and
    kx INSIDE the NEFF; marginal = (median T(k) - median T(1)) / (k-1) =
    pure HW time (per-call dispatch overhead and tunnel latency cancel)."""
    import time
    import jax
    from jax.sharding import NamedSharding, PartitionSpec

    bk = dict(build_kwargs or {})
    if use_v2:
        pre = preprocess_v2(vals, mEmbed, inter, row_idx, col_idx)
        ckey = pre[0]
        per_core = _in_maps_v2(pre)
        builder = lambda repeat: build_nc_v2(ckey, repeat=repeat, **bk)
    else:
        pre = preprocess(vals, mEmbed, inter, row_idx, col_idx)
        C0, C1 = pre[0], pre[1]
        ckey = (C0, C1)
        per_core = _in_maps(pre)
        builder = lambda repeat: build_nc(C0, C1, repeat=repeat, **bk)

    def make_runner(repeat):
        ck = (use_v2, ckey, repeat, tuple(sorted(bk.items())))
        if ck not in _NC_CACHE:
            _NC_CACHE[ck] = builder(repeat)
        nc = _NC_CACHE[ck]
        sharded, mesh, in_names, out_names, zero_outs = _make_sharded(nc)
        sh = NamedSharding(mesh, PartitionSpec("core"))
        shapes = {}
        for alloc in nc.m.functions[0].allocations:
            if isinstance(alloc, mybir.MemoryLocationSet) and alloc.kind == "ExternalInput":
                shapes[alloc.memorylocations[0].name] = (
                    tuple(alloc.tensor_shape), mybir.dt.np(alloc.dtype))
        def get_in(c, n):
            if n in per_core[c]:
                return np.asarray(per_core[c][n])
            shp, dt = shapes[n]
            return np.zeros(shp, dt)
        concat_in = [
            jax.device_put(
                np.concatenate([get_in(c, n) for c in range(NCORES)], axis=0), sh)
            for n in in_names
        ]
        concat_zero = [
            jax.device_put(np.zeros((NCORES * z.shape[0], *z.shape[1:]), z.dtype), sh)
            for z in zero_outs
        ]
        def run():
            out = sharded(*concat_in, *concat_zero)
            jax.block_until_ready(out)
        run()   # warm-up / compile
        return run

    r1 = make_runner(1)
    rk = make_runner(k)

    # Model switches cost ~10 ms through the tunnel, so time each NEFF in
    # bursts of consecutive calls (drop the first two after each switch) and
    # alternate bursts to cancel slow drift in the per-call overhead.
    def burst(run, n=6, discard=2):
        ts = []
        for _ in range(n):
            t0 = time.perf_counter(); run(); ts.append(time.perf_counter() - t0)
        return ts[discard:]

    t1s, tks = [], []
    for _ in range(max(6, samples // 2)):
        t1s += burst(r1)
        tks += burst(rk)
    t1 = float(np.median(t1s))
    tk = float(np.median(tks))
    marginal_ns = (tk - t1) / (k - 1) * 1e9
    return int(marginal_ns), int(t1 * 1e9), int(tk * 1e9)

